# revision 24
# baseline (speedup 1.0000x reference)
"""Trainium2 Bass kernel for nn_Adapter_30674656428557 (GNN message passing).

Strategy (8 NeuronCores, SPMD, no collectives):
  - Nodes sharded by range: core c owns nodes [c*6250, (c+1)*6250).
  - Edges sharded by source node, so every core computes its nodes'
    scatter-mean fully locally.
  - Per core, nodes are processed in 7 uniform chunks of 896 (no tiny
    tail phase).  Within each chunk nodes are degree-sorted and edges
    laid out level-major: level j holds the j-th edge-PAIR of every
    node that has one.  Level capacities are computed exactly from the
    data (max over all 56 core-chunks), so padding is minimal and the
    compiled graph is shared by all cores.
  - edge_attr is shipped as fp8 e4m3, pre-scaled by 32/deg (relu is
    positively homogeneous, so the scatter sums are 32x the means; the
    1/32 is folded into W_fusion's ntf rows).  Halves attr DMA traffic.
  - Device, per chunk:
      time_feat: fp8 matmuls, even/odd pair streams on separate PE
                 row-groups, [128,1024] 2-bank PSUM tiles, relu
                 evacuation in 1024-wide ops alternating DVE/ScalarE.
      scatter  : PSUM-accumulated matmuls over levels with the
                 stacked-identity [I64; I64] fold; even-node and
                 odd-node accumulators in separate banks, concurrent
                 via PE column groups.
      MLP      : even/odd 448-node blocks packed into partition halves
                 of shared [128,448] PSUM banks so every activation /
                 matmul runs at full 128-lane width; up-projection
                 even/odd streams run on separate PE row groups.
      residual + b_up fused into the output evacuation (DVE stt);
      output stored as bf16 (halves out DMA traffic).
  - A burst of dummy warm-up matmuls at graph start keeps the PE HAM
    clock-gate warm through the initial DMA window.
"""

import math
import sys
from contextlib import ExitStack

import numpy as np

sys.path.insert(0, "/opt/trn_rl_repo")

from concourse import bacc, mybir, tile  # noqa: E402
from concourse.bass_utils import run_bass_kernel_spmd  # noqa: E402

DT = mybir.dt
BF = DT.bfloat16
F32 = DT.float32
FP8 = DT.float8e4
NPBF = DT.np(BF)
NPF8 = DT.np(FP8)

N_NODES = 50000
N_EDGES = 1600000
IN_CH = 256
ADAPTER = 64
EDGE_DIM = 32

NCORES = 8
NC_NODES = N_NODES // NCORES     # 6250
NCH = 7
CHUNK = 896                      # 7*896 = 6272 >= 6250
BLK = CHUNK // 2                 # 448
N_STORE = NCH * CHUNK            # 6272
XT_COLS = 4 * BLK                # 1792: [e_h0 | o_h0 | e_h1 | o_h1]
STRIP = 1024

_GRAPH_CACHE = {}


def _compute_caps(deg):
    """Exact per-level pair capacities, shared by all core-chunks.

    cap_j = max over all (core, chunk) of #nodes in the chunk with
    deg > 2j (i.e. having a j-th edge pair).  caps[0] is forced to
    CHUNK so the level-0 matmul initializes every accumulator column.
    """
    caps = np.zeros(512, dtype=np.int64)
    for c in range(NCORES):
        for ch in range(NCH):
            lo = c * NC_NODES + ch * CHUNK
            hi = min(c * NC_NODES + (ch + 1) * CHUNK, (c + 1) * NC_NODES)
            d = np.sort(deg[lo:hi])[::-1]
            pairs = (d + 1) // 2
            mx = int(pairs[0]) if len(pairs) else 0
            for j in range(mx):
                caps[j] = max(caps[j], int(np.searchsorted(-pairs, -j, side="left")))
    nlev = int(np.max(np.nonzero(caps)) + 1) if caps.any() else 1
    caps = caps[:nlev]
    caps[0] = CHUNK
    # monotone non-increasing (true by construction, enforce anyway)
    for j in range(1, nlev):
        caps[j] = min(caps[j], caps[j - 1])
    return [int(x) for x in caps]


def _group_levels(caps):
    """Batch levels into equal-width groups so one matmul (zero-stride
    PSUM out AP) accumulates several levels.  Returns list of
    (t, w, base) groups; level j of group g occupies tf columns
    [base + i*w, base + i*w + w).  Total rhs free size t*w <= 1024
    (bf16 moving-operand limit)."""
    nA = [(c + 1) // 2 for c in caps]
    groups = []
    base = 0
    for w in nA:
        groups.append((1, w, base))
        base += w
    return groups


def _layout(caps):
    """Per-level (width, base) derived from the groups + total columns.

    The column count is padded to a multiple of 16 so the DoubleRow
    pair-slab stride (C16) satisfies the ISA step%16==0 constraint.
    """
    groups = _group_levels(caps)
    widths = []
    lbase = []
    for t, w, base in groups:
        for i in range(t):
            widths.append(w)
            lbase.append(base + i * w)
    cch = groups[-1][0] * groups[-1][1] + groups[-1][2]
    c16 = (cch + 15) // 16 * 16
    return widths, np.array(lbase, dtype=np.int64), c16


def _build_graph(caps):
    key = tuple(caps)
    if key in _GRAPH_CACHE:
        return _GRAPH_CACHE[key]

    widths, lbase, C_CH = _layout(caps)
    groups = _group_levels(caps)

    nc = bacc.Bacc("TRN2", target_bir_lowering=False, debug=False,
                   num_devices=NCORES)

    attr_d = nc.dram_tensor("attr2", [NCH * 128, C_CH], FP8,
                            kind="ExternalInput").ap()
    xt_d = nc.dram_tensor("xt", [NCH * 128, XT_COLS], BF,
                          kind="ExternalInput").ap()
    w2_d = nc.dram_tensor("w2", [128, 64], FP8, kind="ExternalInput").ap()
    wd_d = nc.dram_tensor("wd", [128, 128], BF, kind="ExternalInput").ap()
    wf_d = nc.dram_tensor("wf", [128, 64], BF, kind="ExternalInput").ap()
    wu_d = nc.dram_tensor("wu", [128, 256], BF, kind="ExternalInput").ap()
    fold_d = nc.dram_tensor("fold", [128, 128], FP8, kind="ExternalInput").ap()
    bias_d = nc.dram_tensor("biases", [128, 4], F32, kind="ExternalInput").ap()
    out_d = nc.dram_tensor("out", [NCH * 128, XT_COLS], BF,
                           kind="ExternalOutput").ap()

    Relu = mybir.ActivationFunctionType.Relu

    with tile.TileContext(nc) as tc, ExitStack() as ctx:
        consts = ctx.enter_context(tc.tile_pool(name="consts", bufs=1))
        attr_pool = ctx.enter_context(tc.tile_pool(name="attr", bufs=3))
        xt_pool = ctx.enter_context(tc.tile_pool(name="xtp", bufs=3))
        tf_pool = ctx.enter_context(tc.tile_pool(name="tf", bufs=2))
        fin_pool = ctx.enter_context(tc.tile_pool(name="fin", bufs=4))
        fus_pool = ctx.enter_context(tc.tile_pool(name="fus", bufs=2))
        outp = ctx.enter_context(tc.tile_pool(name="outp", bufs=3))
        ps_tf = ctx.enter_context(tc.tile_pool(name="ps_tf", bufs=2, space="PSUM"))
        ps_acc = ctx.enter_context(tc.tile_pool(name="ps_acc", bufs=2, space="PSUM"))
        ps_mlp = ctx.enter_context(tc.tile_pool(name="ps_mlp", bufs=2, space="PSUM"))

        # consts go on the scalar HWDGE queue so the sync queue's first
        # DMA is chunk 0's attr (critical path).
        w2 = consts.tile([128, 64], FP8)
        nc.scalar.dma_start(w2[:], w2_d[:])
        wd = consts.tile([128, 128], BF)
        nc.scalar.dma_start(wd[:], wd_d[:])
        wf = consts.tile([128, 64], BF)
        nc.scalar.dma_start(wf[:], wf_d[:])
        wu = consts.tile([128, 256], BF)
        nc.scalar.dma_start(wu[:], wu_d[:])
        fold = consts.tile([128, 128], FP8)
        nc.scalar.dma_start(fold[:], fold_d[:])
        biases = consts.tile([128, 4], F32)
        nc.scalar.dma_start(biases[:], bias_d[:])

        # PE warm-up: dependency-free matmuls on a zeroed tile keep the
        # HAM clock-gate busy through the initial DMA window.
        warm = consts.tile([128, 512], BF)
        nc.gpsimd.memset(warm[:], 0.0)
        pw = ps_tf.tile([128, STRIP], F32, tag="ps_tf")
        for wi in range(18):
            nc.tensor.matmul(pw[0:64, 0:384], warm[:, 0:64], warm[:, 0:384])

        DRmode = mybir.MatmulPerfMode.DoubleRow
        Ident = mybir.ActivationFunctionType.Identity
        fold_lo = fold[0:64, :].rearrange("k (t m) -> k t m", t=2)
        fold_hi = fold[64:128, :].rearrange("k (t m) -> k t m", t=2)
        ng = len(groups)
        nstrips = (C_CH + STRIP - 1) // STRIP
        state = {}
        flip = [0]

        def emit_tf(ch):
            """Per-strip tf MMs + evacs for chunk ch (one yield per strip).

            tf layout: fp8, slot s feats of column c at [0:64 or 64:128,
            (s%2)*C_CH + c] — slot0/1 = even pair in the lower partition
            half as two column slabs, slot2/3 = odd pair in the upper
            half.  The slab pitch C_CH feeds the DoubleRow pair fold.
            Four independent K=32 streams on PE quadrant positions
            (0,0) (32,0) (64,64) (96,64) — disjoint subarrays, so each
            stream has its own weight-buffer pair.
            """
            r0 = ch * 128
            attr = attr_pool.tile([128, C_CH], FP8, tag="attr")
            nc.sync.dma_start(attr[:], attr_d[r0:r0 + 128, :])
            xt = xt_pool.tile([128, XT_COLS], BF, tag="xt")
            nc.sync.dma_start(xt[:], xt_d[r0:r0 + 128, :])
            tf = tf_pool.tile([128, 2 * C_CH], FP8, tag="tf")
            state[ch] = (tf, xt)
            for si in range(nstrips):
                s0 = si * STRIP
                w_ = min(STRIP, C_CH - s0)
                pA = ps_tf.tile([128, STRIP], F32, tag="ps_tf")
                pB = ps_tf.tile([128, STRIP], F32, tag="ps_tf")
                for x in range(0, w_, 512):
                    hw = min(512, w_ - x)
                    sl = slice(s0 + x, s0 + x + hw)
                    nc.tensor.matmul(pA[0:64, x:x + hw], w2[0:32, :],
                                     attr[0:32, sl])
                    nc.tensor.matmul(pB[0:64, x:x + hw], w2[32:64, :],
                                     attr[32:64, sl])
                    nc.tensor.matmul(pA[64:128, x:x + hw], w2[64:96, :],
                                     attr[64:96, sl])
                    nc.tensor.matmul(pB[64:128, x:x + hw], w2[96:128, :],
                                     attr[96:128, sl], tile_position=(96, 64))
                # relu evacuation: one strip half on each engine so the
                # per-strip chain stays parallel.
                dst_a = tf[:, s0:s0 + w_]
                dst_b = tf[:, C_CH + s0:C_CH + s0 + w_]
                if flip[0] % 2 == 0:
                    nc.vector.tensor_scalar_max(dst_a, pA[:, 0:w_], 0.0)
                    nc.scalar.activation(dst_b, pB[:, 0:w_], Relu)
                else:
                    nc.scalar.activation(dst_a, pA[:, 0:w_], Relu)
                    nc.vector.tensor_scalar_max(dst_b, pB[:, 0:w_], 0.0)
                flip[0] += 1
                yield

        def filler(acc_e, n):
            """PE warmth filler: matmuls on the one quadrant no real work
            uses (rows 0-63 operands -> out partitions 64-127, position
            (0,64)), writing the unused upper half of an acc bank with
            start=False so the live accumulation's has_written bits are
            untouched.  Nearly free when the pipeline is busy; keeps the
            HAM clock-gate from re-throttling when the PE would stall."""
            for _ in range(n):
                nc.tensor.matmul(acc_e[64:128, 0:448], warm[0:64, 0:64],
                                 warm[0:64, 0:448], start=False, stop=False,
                                 skip_group_check=True)

        def emit_tail(ch):
            """Scatter + MLP + store for chunk ch, sliced for interleave.

            Scatter: DoubleRow matmuls fold the pair slabs while
            accumulating over levels; even-node sums from rows 0-63 ->
            acc_e[0:64] (position (0,0)), odd-node sums from rows
            64-127 -> acc_o[0:64] (position (64,0)).
            """
            tf, xt = state.pop(ch)
            r0 = ch * 128
            acc_e = ps_acc.tile([128, 512], F32, tag="acc")
            acc_o = ps_acc.tile([128, 512], F32, tag="acc")
            state["acc"] = acc_e

            # down-projection, hoisted ahead of the scatter: dependency-
            # free PE work that pads out the pipeline while the previous
            # chunk's evacuations drain.
            psn = ps_mlp.tile([128, 512], F32, tag="mlp")
            psn2 = ps_mlp.tile([128, 512], F32, tag="mlp")
            nc.tensor.matmul(psn[64:128, 0:BLK], wd[:, 0:64], xt[:, 0:BLK],
                             start=True, stop=False)
            nc.tensor.matmul(psn[64:128, 0:BLK], wd[:, 64:128],
                             xt[:, 2 * BLK:3 * BLK], start=False, stop=True)
            nc.tensor.matmul(psn2[64:128, 0:BLK], wd[:, 0:64],
                             xt[:, BLK:2 * BLK], start=True, stop=False)
            nc.tensor.matmul(psn2[64:128, 0:BLK], wd[:, 64:128],
                             xt[:, 3 * BLK:4 * BLK], start=False, stop=True)
            fin_e = fin_pool.tile([128, BLK], BF, tag="fin")
            fin_o = fin_pool.tile([128, BLK], BF, tag="fin")
            nc.scalar.activation(fin_e[64:128, :], psn[64:128, 0:BLK], Relu,
                                 bias=biases[64:128, 0:1])
            nc.scalar.activation(fin_o[64:128, :], psn2[64:128, 0:BLK], Relu,
                                 bias=biases[64:128, 0:1])
            yield

            tf3_lo = tf[0:64, :].rearrange("p (t c) -> p t c", t=2)
            tf3_hi = tf[64:128, :].rearrange("p (t c) -> p t c", t=2)
            for g, (t, w, base) in enumerate(groups):
                nc.tensor.matmul(acc_e[0:64, 0:w], fold_lo,
                                 tf3_lo[:, :, base:base + w],
                                 start=(g == 0), stop=(g == ng - 1),
                                 perf_mode=DRmode)
                nc.tensor.matmul(acc_o[0:64, 0:w], fold_hi,
                                 tf3_hi[:, :, base:base + w],
                                 start=(g == 0), stop=(g == ng - 1),
                                 perf_mode=DRmode, tile_position=(64, 0))
                if g % 5 == 4:
                    yield

            # fin_e = [ntf_e(0:64); nf_e(64:128)], fin_o likewise
            nc.scalar.activation(fin_e[0:64, :], acc_e[0:64, 0:BLK], Ident)
            nc.scalar.activation(fin_o[0:64, :], acc_o[0:64, 0:BLK], Ident)
            yield

            # fusion: one K=128 matmul per parity block, same weights
            psf = ps_mlp.tile([128, 512], F32, tag="mlp")
            nc.tensor.matmul(psf[0:64, 0:BLK], wf[:, 0:64], fin_e[:, :])
            nc.tensor.matmul(psf[64:128, 0:BLK], wf[:, 0:64], fin_o[:, :])
            fused = fus_pool.tile([128, BLK], BF, tag="fused")
            nc.scalar.activation(fused[:, :], psf[:, 0:BLK], Relu,
                                 bias=biases[:, 1:2])
            yield

            # up-projection + residual + bias, output bf16
            ob = outp.tile([128, XT_COLS], BF, tag="ob")
            for h in range(2):
                psu_e = ps_mlp.tile([128, 512], F32, tag="mlp")
                psu_o = ps_mlp.tile([128, 512], F32, tag="mlp")
                nc.tensor.matmul(psu_e[:, 0:BLK],
                                 wu[0:64, 128 * h:128 * (h + 1)], fused[0:64, :])
                nc.tensor.matmul(psu_o[:, 0:BLK],
                                 wu[64:128, 128 * h:128 * (h + 1)],
                                 fused[64:128, :])
                e0 = 2 * BLK * h
                nc.vector.scalar_tensor_tensor(
                    ob[:, e0:e0 + BLK], psu_e[:, 0:BLK], biases[:, 2 + h:3 + h],
                    xt[:, e0:e0 + BLK],
                    op0=mybir.AluOpType.add, op1=mybir.AluOpType.add)
                nc.vector.scalar_tensor_tensor(
                    ob[:, e0 + BLK:e0 + 2 * BLK], psu_o[:, 0:BLK],
                    biases[:, 2 + h:3 + h], xt[:, e0 + BLK:e0 + 2 * BLK],
                    op0=mybir.AluOpType.add, op1=mybir.AluOpType.add)
                yield
            # stores go out on the gpsimd SWDGE queue: keeps the sync
            # HWDGE queue free for the latency-critical attr/xt loads
            nc.gpsimd.dma_start(out_d[r0:r0 + 128, :], ob[:])

        # software pipeline: chunk c's scatter+MLP interleaves with chunk
        # c+1's tf strips, so the PE instruction stream stays dense while
        # DVE/ScalarE drain the next chunk's strips in parallel.
        for ch in range(NCH + 1):
            tf_gen = emit_tf(ch) if ch < NCH else None
            sc_gen = emit_tail(ch - 1) if ch >= 1 else None
            while tf_gen is not None or sc_gen is not None:
                if tf_gen is not None:
                    try:
                        next(tf_gen)
                    except StopIteration:
                        tf_gen = None
                if sc_gen is not None:
                    try:
                        next(sc_gen)
                    except StopIteration:
                        sc_gen = None
                acc = state.get("acc")
                if acc is not None:
                    filler(acc, 2)

    nc.compile()
    _GRAPH_CACHE[key] = nc
    return nc


def prepare(x, edge_index, edge_attr, W_down, b_down, W_time, b_time,
            W_fusion, b_fusion, W_up, b_up):
    """Host-side sharding/layout.

    Returns (caps, in_maps, node_cols[NCORES, NC_NODES, 2]) where
    node_cols[c, n] = (row_block, col) into the per-core out tensor.
    """
    x = np.asarray(x, dtype=np.float32)
    edge_index = np.asarray(edge_index)
    edge_attr = np.asarray(edge_attr, dtype=np.float32)
    W_down = np.asarray(W_down, dtype=np.float32)
    b_down = np.asarray(b_down, dtype=np.float32)
    W_time = np.asarray(W_time, dtype=np.float32)
    b_time = np.asarray(b_time, dtype=np.float32)
    W_fusion = np.asarray(W_fusion, dtype=np.float32)
    b_fusion = np.asarray(b_fusion, dtype=np.float32)
    W_up = np.asarray(W_up, dtype=np.float32)
    b_up = np.asarray(b_up, dtype=np.float32)

    assert not np.any(b_time), "ghost slots in the padded layout assume b_time == 0"

    src = edge_index[0].astype(np.int64)
    deg = np.bincount(src, minlength=N_NODES).astype(np.int64)

    caps = _compute_caps(deg)
    widths, lbase, C_CH = _layout(caps)

    # per-node: within-chunk degree-sorted position
    s_pos = np.empty(N_NODES, dtype=np.int64)
    for c in range(NCORES):
        for ch in range(NCH):
            lo = c * NC_NODES + ch * CHUNK
            hi = min(c * NC_NODES + (ch + 1) * CHUNK, (c + 1) * NC_NODES)
            order = np.argsort(-deg[lo:hi], kind="stable")
            s = np.empty(hi - lo, dtype=np.int64)
            s[order] = np.arange(hi - lo)
            s_pos[lo:hi] = s
    ln = np.arange(N_NODES) % NC_NODES
    chn = np.minimum(ln // CHUNK, NCH - 1)

    # per-edge placement: slot = (col, halfq, par) within the chunk
    esort = np.argsort(src, kind="stable")
    starts = np.zeros(N_NODES + 1, dtype=np.int64)
    np.cumsum(deg, out=starts[1:])
    srcs = src[esort]
    rank = np.arange(N_EDGES, dtype=np.int64) - starts[srcs]
    q = rank // 2                     # pair (level) index
    par = rank % 2
    colc = lbase[q] + s_pos[srcs] // 2
    halfq = s_pos[srcs] % 2
    slot = colc * 4 + halfq * 2 + par
    core_of_edge = srcs // NC_NODES

    # fp8 attr, pre-scaled by 32/deg (relu is positively homogeneous)
    recall = (32.0 / np.maximum(deg, 1)).astype(np.float32)
    ea = np.clip(edge_attr * recall[src][:, None], -240.0, 240.0).astype(NPF8)

    # shared weights: w2 = W_time.T replicated per 32-partition slot
    w2 = np.concatenate([W_time.T] * 4, axis=0).astype(NPF8)            # [128,64]
    wd = np.concatenate([W_down.T[0:128], W_down.T[128:256]], axis=1).astype(NPBF)
    wfT = W_fusion.T.copy()                                             # [128, 64]
    wfT[64:128] *= 1.0 / 32.0         # compensate the 32/deg pre-scale
    # fin = [ntf(0:64); nf(64:128)] for both parities
    wf = np.concatenate([wfT[64:128], wfT[0:64]], axis=0).astype(NPBF)  # [128,64]
    wu = np.concatenate([W_up.T, W_up.T], axis=0).astype(NPBF)          # [128,256]
    fold = np.concatenate([np.eye(64), np.eye(64)], axis=1)             # [64,128]
    fold = np.concatenate([fold, fold], axis=0).astype(NPF8)            # [128,128]
    biases = np.zeros((128, 4), dtype=np.float32)
    biases[0:64, 0] = b_down
    biases[64:128, 0] = b_down
    biases[0:64, 1] = b_fusion
    biases[64:128, 1] = b_fusion
    biases[:, 2] = b_up[0:128]
    biases[:, 3] = b_up[128:256]

    in_maps = []
    node_cols = np.empty((NCORES, NC_NODES), dtype=np.int64)
    for c in range(NCORES):
        em = core_of_edge == c
        attr_flat = np.zeros((NCH, C_CH * 4, EDGE_DIM), dtype=NPF8)
        ech = chn[srcs[em]]
        attr_flat[ech, slot[em]] = ea[esort[em]]
        # [NCH, C*4, 32] -> per chunk [128, C]: partition = slot*32+dim
        attr2 = np.ascontiguousarray(
            attr_flat.reshape(NCH, C_CH, 4, EDGE_DIM)
            .transpose(0, 2, 3, 1).reshape(NCH * 128, C_CH))

        # x layout: per chunk cols [e_h0 | o_h0 | e_h1 | o_h1]
        nlo = c * NC_NODES
        lpos = s_pos[nlo:nlo + NC_NODES]
        lch = chn[nlo:nlo + NC_NODES]
        colq = lpos // 2
        colp = lpos % 2
        node_cols[c] = lch * XT_COLS * 0 + colp * BLK + colq  # col within h-block
        xt = np.zeros((NCH * 128, XT_COLS), dtype=np.float32)
        xv = x[nlo:nlo + NC_NODES]                            # [6250, 256]
        rows = lch * 128
        # scatter x into layout
        for h in range(2):
            base = 2 * BLK * h
            cols = base + colp * BLK + colq
            for d in range(128):
                xt[rows + d, cols] = xv[:, 128 * h + d]
        in_maps.append({
            "attr2": attr2,
            "xt": xt.astype(NPBF),
            "w2": w2,
            "wd": wd,
            "wf": wf,
            "wu": wu,
            "fold": fold,
            "biases": biases,
        })
    return caps, in_maps, node_cols


def run(caps, in_maps, trace=False, **kw):
    nc = _build_graph(caps)
    return run_bass_kernel_spmd(nc, in_maps, core_ids=list(range(NCORES)),
                                trace=trace, **kw)


def unshard(results, node_cols):
    out = np.empty((N_NODES, IN_CH), dtype=np.float32)
    lch = np.minimum(np.arange(NC_NODES) // CHUNK, NCH - 1)
    rows = lch * 128
    for c in range(NCORES):
        o = np.asarray(results[c]["out"], dtype=np.float32)   # [NCH*128, XT_COLS]
        cols = node_cols[c]
        for h in range(2):
            colh = 2 * BLK * h + cols
            for d in range(128):
                out[c * NC_NODES:(c + 1) * NC_NODES, 128 * h + d] = \
                    o[rows + d, colh]
    return out


def kernel(**inputs):
    caps, in_maps, node_cols = prepare(**inputs)
    res = run(caps, in_maps, trace=False)
    return unshard(res.results, node_cols)


# revision 25
# speedup vs baseline: 1.1628x; 1.1628x over previous
"""Trainium2 Bass kernel for nn_Adapter_30674656428557 (GNN message passing).

Strategy (8 NeuronCores, SPMD, no collectives):
  - Nodes sharded by range: core c owns nodes [c*6250, (c+1)*6250).
  - Edges sharded by source node, so every core computes its nodes'
    scatter-mean fully locally.
  - Per core, nodes are processed in 7 uniform chunks of 896 (no tiny
    tail phase).  Within each chunk nodes are degree-sorted and edges
    laid out level-major: level j holds the j-th edge-PAIR of every
    node that has one.  Level capacities are computed exactly from the
    data (max over all 56 core-chunks), so padding is minimal and the
    compiled graph is shared by all cores.
  - edge_attr is shipped as fp8 e4m3, pre-scaled by 32/deg (relu is
    positively homogeneous, so the scatter sums are 32x the means; the
    1/32 is folded into W_fusion's ntf rows).  Halves attr DMA traffic.
  - Device, per chunk:
      time_feat: fp8 matmuls, even/odd pair streams on separate PE
                 row-groups, [128,1024] 2-bank PSUM tiles, relu
                 evacuation in 1024-wide ops alternating DVE/ScalarE.
      scatter  : PSUM-accumulated matmuls over levels with the
                 stacked-identity [I64; I64] fold; even-node and
                 odd-node accumulators in separate banks, concurrent
                 via PE column groups.
      MLP      : even/odd 448-node blocks packed into partition halves
                 of shared [128,448] PSUM banks so every activation /
                 matmul runs at full 128-lane width; up-projection
                 even/odd streams run on separate PE row groups.
      residual + b_up fused into the output evacuation (DVE stt);
      output stored as bf16 (halves out DMA traffic).
  - A burst of dummy warm-up matmuls at graph start keeps the PE HAM
    clock-gate warm through the initial DMA window.
"""

import math
import sys
from contextlib import ExitStack

import numpy as np

sys.path.insert(0, "/opt/trn_rl_repo")

from concourse import bacc, mybir, tile  # noqa: E402
from concourse.bass_utils import run_bass_kernel_spmd  # noqa: E402

DT = mybir.dt
BF = DT.bfloat16
F32 = DT.float32
FP8 = DT.float8e4
NPBF = DT.np(BF)
NPF8 = DT.np(FP8)

N_NODES = 50000
N_EDGES = 1600000
IN_CH = 256
ADAPTER = 64
EDGE_DIM = 32

NCORES = 8
NC_NODES = N_NODES // NCORES     # 6250
NCH = 7
CHUNK = 896                      # 7*896 = 6272 >= 6250
BLK = CHUNK // 2                 # 448
N_STORE = NCH * CHUNK            # 6272
XT_COLS = 4 * BLK                # 1792: [e_h0 | o_h0 | e_h1 | o_h1]
STRIP = 1024

_GRAPH_CACHE = {}


def _compute_caps(deg):
    """Exact per-level pair capacities, shared by all core-chunks.

    cap_j = max over all (core, chunk) of #nodes in the chunk with
    deg > 2j (i.e. having a j-th edge pair).  caps[0] is forced to
    CHUNK so the level-0 matmul initializes every accumulator column.
    """
    caps = np.zeros(512, dtype=np.int64)
    for c in range(NCORES):
        for ch in range(NCH):
            lo = c * NC_NODES + ch * CHUNK
            hi = min(c * NC_NODES + (ch + 1) * CHUNK, (c + 1) * NC_NODES)
            d = np.sort(deg[lo:hi])[::-1]
            pairs = (d + 1) // 2
            mx = int(pairs[0]) if len(pairs) else 0
            for j in range(mx):
                caps[j] = max(caps[j], int(np.searchsorted(-pairs, -j, side="left")))
    nlev = int(np.max(np.nonzero(caps)) + 1) if caps.any() else 1
    caps = caps[:nlev]
    caps[0] = CHUNK
    # monotone non-increasing (true by construction, enforce anyway)
    for j in range(1, nlev):
        caps[j] = min(caps[j], caps[j - 1])
    return [int(x) for x in caps]


def _group_levels(caps):
    """Batch levels into equal-width groups so one matmul (zero-stride
    PSUM out AP) accumulates several levels.  Returns list of
    (t, w, base) groups; level j of group g occupies tf columns
    [base + i*w, base + i*w + w).  Total rhs free size t*w <= 1024
    (bf16 moving-operand limit)."""
    nA = [(c + 1) // 2 for c in caps]
    groups = []
    base = 0
    for w in nA:
        groups.append((1, w, base))
        base += w
    return groups


def _layout(caps):
    """Per-level (width, base) derived from the groups + total columns.

    The column count is padded to a multiple of 16 so the DoubleRow
    pair-slab stride (C16) satisfies the ISA step%16==0 constraint.
    """
    groups = _group_levels(caps)
    widths = []
    lbase = []
    for t, w, base in groups:
        for i in range(t):
            widths.append(w)
            lbase.append(base + i * w)
    cch = groups[-1][0] * groups[-1][1] + groups[-1][2]
    c16 = (cch + 15) // 16 * 16
    return widths, np.array(lbase, dtype=np.int64), c16


def _build_graph(caps):
    key = tuple(caps)
    if key in _GRAPH_CACHE:
        return _GRAPH_CACHE[key]

    widths, lbase, C_CH = _layout(caps)
    groups = _group_levels(caps)

    nc = bacc.Bacc("TRN2", target_bir_lowering=False, debug=False,
                   num_devices=NCORES)

    attr_d = nc.dram_tensor("attr2", [NCH * 128, C_CH], FP8,
                            kind="ExternalInput").ap()
    xt_d = nc.dram_tensor("xt", [NCH * 128, XT_COLS], BF,
                          kind="ExternalInput").ap()
    w2_d = nc.dram_tensor("w2", [128, 64], FP8, kind="ExternalInput").ap()
    wd_d = nc.dram_tensor("wd", [128, 128], BF, kind="ExternalInput").ap()
    wf_d = nc.dram_tensor("wf", [128, 64], BF, kind="ExternalInput").ap()
    wu_d = nc.dram_tensor("wu", [128, 256], BF, kind="ExternalInput").ap()
    fold_d = nc.dram_tensor("fold", [128, 128], FP8, kind="ExternalInput").ap()
    bias_d = nc.dram_tensor("biases", [128, 4], F32, kind="ExternalInput").ap()
    out_d = nc.dram_tensor("out", [NCH * 128, XT_COLS], BF,
                           kind="ExternalOutput").ap()

    Relu = mybir.ActivationFunctionType.Relu

    with tile.TileContext(nc) as tc, ExitStack() as ctx:
        consts = ctx.enter_context(tc.tile_pool(name="consts", bufs=1))
        attr_pool = ctx.enter_context(tc.tile_pool(name="attr", bufs=3))
        xt_pool = ctx.enter_context(tc.tile_pool(name="xtp", bufs=3))
        tf_pool = ctx.enter_context(tc.tile_pool(name="tf", bufs=2))
        fin_pool = ctx.enter_context(tc.tile_pool(name="fin", bufs=4))
        fus_pool = ctx.enter_context(tc.tile_pool(name="fus", bufs=2))
        outp = ctx.enter_context(tc.tile_pool(name="outp", bufs=3))
        ps_tf = ctx.enter_context(tc.tile_pool(name="ps_tf", bufs=2, space="PSUM"))
        ps_acc = ctx.enter_context(tc.tile_pool(name="ps_acc", bufs=2, space="PSUM"))
        ps_mlp = ctx.enter_context(tc.tile_pool(name="ps_mlp", bufs=2, space="PSUM"))

        # consts go on the scalar HWDGE queue so the sync queue's first
        # DMA is chunk 0's attr (critical path).
        w2 = consts.tile([128, 64], FP8)
        nc.scalar.dma_start(w2[:], w2_d[:])
        wd = consts.tile([128, 128], BF)
        nc.scalar.dma_start(wd[:], wd_d[:])
        wf = consts.tile([128, 64], BF)
        nc.scalar.dma_start(wf[:], wf_d[:])
        wu = consts.tile([128, 256], BF)
        nc.scalar.dma_start(wu[:], wu_d[:])
        fold = consts.tile([128, 128], FP8)
        nc.scalar.dma_start(fold[:], fold_d[:])
        biases = consts.tile([128, 4], F32)
        nc.scalar.dma_start(biases[:], bias_d[:])

        # PE warm-up: dependency-free matmuls on a zeroed tile keep the
        # HAM clock-gate busy through the initial DMA window.
        warm = consts.tile([128, 512], BF)
        nc.gpsimd.memset(warm[:], 0.0)
        pw = ps_tf.tile([128, STRIP], F32, tag="ps_tf")
        for wi in range(18):
            nc.tensor.matmul(pw[0:64, 0:384], warm[:, 0:64], warm[:, 0:384])

        DRmode = mybir.MatmulPerfMode.DoubleRow
        Ident = mybir.ActivationFunctionType.Identity
        fold_lo = fold[0:64, :].rearrange("k (t m) -> k t m", t=2)
        fold_hi = fold[64:128, :].rearrange("k (t m) -> k t m", t=2)
        ng = len(groups)
        nstrips = (C_CH + STRIP - 1) // STRIP
        state = {}
        flip = [0]

        def emit_tf(ch):
            """Per-strip tf MMs + evacs for chunk ch (one yield per strip).

            tf layout: fp8, slot s feats of column c at [0:64 or 64:128,
            (s%2)*C_CH + c] — slot0/1 = even pair in the lower partition
            half as two column slabs, slot2/3 = odd pair in the upper
            half.  The slab pitch C_CH feeds the DoubleRow pair fold.
            Four independent K=32 streams on PE quadrant positions
            (0,0) (32,0) (64,64) (96,64) — disjoint subarrays, so each
            stream has its own weight-buffer pair.
            """
            r0 = ch * 128
            attr = attr_pool.tile([128, C_CH], FP8, tag="attr")
            nc.sync.dma_start(attr[:], attr_d[r0:r0 + 128, :])
            xt = xt_pool.tile([128, XT_COLS], BF, tag="xt")
            nc.sync.dma_start(xt[:], xt_d[r0:r0 + 128, :])
            tf = tf_pool.tile([128, 2 * C_CH], FP8, tag="tf")
            state[ch] = (tf, xt)
            for si in range(nstrips):
                s0 = si * STRIP
                w_ = min(STRIP, C_CH - s0)
                pA = ps_tf.tile([128, STRIP], F32, tag="ps_tf")
                pB = ps_tf.tile([128, STRIP], F32, tag="ps_tf")
                for x in range(0, w_, 512):
                    hw = min(512, w_ - x)
                    sl = slice(s0 + x, s0 + x + hw)
                    nc.tensor.matmul(pA[0:64, x:x + hw], w2[0:32, :],
                                     attr[0:32, sl])
                    nc.tensor.matmul(pB[0:64, x:x + hw], w2[32:64, :],
                                     attr[32:64, sl])
                    nc.tensor.matmul(pA[64:128, x:x + hw], w2[64:96, :],
                                     attr[64:96, sl])
                    nc.tensor.matmul(pB[64:128, x:x + hw], w2[96:128, :],
                                     attr[96:128, sl], tile_position=(96, 64))
                # relu evacuation: one strip half on each engine so the
                # per-strip chain stays parallel.
                dst_a = tf[:, s0:s0 + w_]
                dst_b = tf[:, C_CH + s0:C_CH + s0 + w_]
                if flip[0] % 2 == 0:
                    nc.vector.tensor_scalar_max(dst_a, pA[:, 0:w_], 0.0)
                    nc.scalar.activation(dst_b, pB[:, 0:w_], Relu)
                else:
                    nc.scalar.activation(dst_a, pA[:, 0:w_], Relu)
                    nc.vector.tensor_scalar_max(dst_b, pB[:, 0:w_], 0.0)
                flip[0] += 1
                yield

        def filler(acc_e, n):
            """PE warmth filler: matmuls on the one quadrant no real work
            uses (rows 0-63 operands -> out partitions 64-127, position
            (0,64)), writing the unused upper half of an acc bank with
            start=False so the live accumulation's has_written bits are
            untouched.  Nearly free when the pipeline is busy; keeps the
            HAM clock-gate from re-throttling when the PE would stall."""
            for _ in range(n):
                nc.tensor.matmul(acc_e[64:128, 0:448], warm[0:64, 0:64],
                                 warm[0:64, 0:448], start=False, stop=False,
                                 skip_group_check=True)

        def emit_tail(ch):
            """Scatter + MLP + store for chunk ch, sliced for interleave.

            Scatter: DoubleRow matmuls fold the pair slabs while
            accumulating over levels; even-node sums from rows 0-63 ->
            acc_e[0:64] (position (0,0)), odd-node sums from rows
            64-127 -> acc_o[0:64] (position (64,0)).
            """
            tf, xt = state.pop(ch)
            r0 = ch * 128
            acc_e = ps_acc.tile([128, 512], F32, tag="acc")
            acc_o = ps_acc.tile([128, 512], F32, tag="acc")
            state["acc"] = acc_e

            # down-projection, hoisted ahead of the scatter: dependency-
            # free PE work that pads out the pipeline while the previous
            # chunk's evacuations drain.
            psn = ps_mlp.tile([128, 512], F32, tag="mlp")
            psn2 = ps_mlp.tile([128, 512], F32, tag="mlp")
            nc.tensor.matmul(psn[64:128, 0:BLK], wd[:, 0:64], xt[:, 0:BLK],
                             start=True, stop=False)
            nc.tensor.matmul(psn[64:128, 0:BLK], wd[:, 64:128],
                             xt[:, 2 * BLK:3 * BLK], start=False, stop=True)
            nc.tensor.matmul(psn2[64:128, 0:BLK], wd[:, 0:64],
                             xt[:, BLK:2 * BLK], start=True, stop=False)
            nc.tensor.matmul(psn2[64:128, 0:BLK], wd[:, 64:128],
                             xt[:, 3 * BLK:4 * BLK], start=False, stop=True)
            fin_e = fin_pool.tile([128, BLK], BF, tag="fin")
            fin_o = fin_pool.tile([128, BLK], BF, tag="fin")
            nc.scalar.activation(fin_e[64:128, :], psn[64:128, 0:BLK], Relu,
                                 bias=biases[64:128, 0:1])
            nc.scalar.activation(fin_o[64:128, :], psn2[64:128, 0:BLK], Relu,
                                 bias=biases[64:128, 0:1])
            yield

            tf3_lo = tf[0:64, :].rearrange("p (t c) -> p t c", t=2)
            tf3_hi = tf[64:128, :].rearrange("p (t c) -> p t c", t=2)
            for g, (t, w, base) in enumerate(groups):
                nc.tensor.matmul(acc_e[0:64, 0:w], fold_lo,
                                 tf3_lo[:, :, base:base + w],
                                 start=(g == 0), stop=(g == ng - 1),
                                 perf_mode=DRmode)
                nc.tensor.matmul(acc_o[0:64, 0:w], fold_hi,
                                 tf3_hi[:, :, base:base + w],
                                 start=(g == 0), stop=(g == ng - 1),
                                 perf_mode=DRmode, tile_position=(64, 0))
                if g % 5 == 4:
                    yield

            # fin_e = [ntf_e(0:64); nf_e(64:128)], fin_o likewise
            nc.scalar.activation(fin_e[0:64, :], acc_e[0:64, 0:BLK], Ident)
            nc.scalar.activation(fin_o[0:64, :], acc_o[0:64, 0:BLK], Ident)
            yield

            # fusion: one K=128 matmul per parity block, same weights
            psf = ps_mlp.tile([128, 512], F32, tag="mlp")
            nc.tensor.matmul(psf[0:64, 0:BLK], wf[:, 0:64], fin_e[:, :])
            nc.tensor.matmul(psf[64:128, 0:BLK], wf[:, 0:64], fin_o[:, :])
            fused = fus_pool.tile([128, BLK], BF, tag="fused")
            nc.scalar.activation(fused[:, :], psf[:, 0:BLK], Relu,
                                 bias=biases[:, 1:2])
            yield

            # up-projection + residual + bias, output bf16
            ob = outp.tile([128, XT_COLS], BF, tag="ob")
            for h in range(2):
                psu_e = ps_mlp.tile([128, 512], F32, tag="mlp")
                psu_o = ps_mlp.tile([128, 512], F32, tag="mlp")
                nc.tensor.matmul(psu_e[:, 0:BLK],
                                 wu[0:64, 128 * h:128 * (h + 1)], fused[0:64, :])
                nc.tensor.matmul(psu_o[:, 0:BLK],
                                 wu[64:128, 128 * h:128 * (h + 1)],
                                 fused[64:128, :])
                e0 = 2 * BLK * h
                nc.vector.scalar_tensor_tensor(
                    ob[:, e0:e0 + BLK], psu_e[:, 0:BLK], biases[:, 2 + h:3 + h],
                    xt[:, e0:e0 + BLK],
                    op0=mybir.AluOpType.add, op1=mybir.AluOpType.add)
                nc.vector.scalar_tensor_tensor(
                    ob[:, e0 + BLK:e0 + 2 * BLK], psu_o[:, 0:BLK],
                    biases[:, 2 + h:3 + h], xt[:, e0 + BLK:e0 + 2 * BLK],
                    op0=mybir.AluOpType.add, op1=mybir.AluOpType.add)
                yield
            # stores go out on the gpsimd SWDGE queue: keeps the sync
            # HWDGE queue free for the latency-critical attr/xt loads
            nc.gpsimd.dma_start(out_d[r0:r0 + 128, :], ob[:])

        # software pipeline: chunk c's scatter+MLP interleaves with chunk
        # c+1's tf strips, so the PE instruction stream stays dense while
        # DVE/ScalarE drain the next chunk's strips in parallel.
        for ch in range(NCH + 1):
            tf_gen = emit_tf(ch) if ch < NCH else None
            sc_gen = emit_tail(ch - 1) if ch >= 1 else None
            while tf_gen is not None or sc_gen is not None:
                if tf_gen is not None:
                    try:
                        next(tf_gen)
                    except StopIteration:
                        tf_gen = None
                if sc_gen is not None:
                    try:
                        next(sc_gen)
                    except StopIteration:
                        sc_gen = None


    nc.compile()
    _GRAPH_CACHE[key] = nc
    return nc


def prepare(x, edge_index, edge_attr, W_down, b_down, W_time, b_time,
            W_fusion, b_fusion, W_up, b_up):
    """Host-side sharding/layout.

    Returns (caps, in_maps, node_cols[NCORES, NC_NODES, 2]) where
    node_cols[c, n] = (row_block, col) into the per-core out tensor.
    """
    x = np.asarray(x, dtype=np.float32)
    edge_index = np.asarray(edge_index)
    edge_attr = np.asarray(edge_attr, dtype=np.float32)
    W_down = np.asarray(W_down, dtype=np.float32)
    b_down = np.asarray(b_down, dtype=np.float32)
    W_time = np.asarray(W_time, dtype=np.float32)
    b_time = np.asarray(b_time, dtype=np.float32)
    W_fusion = np.asarray(W_fusion, dtype=np.float32)
    b_fusion = np.asarray(b_fusion, dtype=np.float32)
    W_up = np.asarray(W_up, dtype=np.float32)
    b_up = np.asarray(b_up, dtype=np.float32)

    assert not np.any(b_time), "ghost slots in the padded layout assume b_time == 0"

    src = edge_index[0].astype(np.int64)
    deg = np.bincount(src, minlength=N_NODES).astype(np.int64)

    caps = _compute_caps(deg)
    widths, lbase, C_CH = _layout(caps)

    # per-node: within-chunk degree-sorted position
    s_pos = np.empty(N_NODES, dtype=np.int64)
    for c in range(NCORES):
        for ch in range(NCH):
            lo = c * NC_NODES + ch * CHUNK
            hi = min(c * NC_NODES + (ch + 1) * CHUNK, (c + 1) * NC_NODES)
            order = np.argsort(-deg[lo:hi], kind="stable")
            s = np.empty(hi - lo, dtype=np.int64)
            s[order] = np.arange(hi - lo)
            s_pos[lo:hi] = s
    ln = np.arange(N_NODES) % NC_NODES
    chn = np.minimum(ln // CHUNK, NCH - 1)

    # per-edge placement: slot = (col, halfq, par) within the chunk
    esort = np.argsort(src, kind="stable")
    starts = np.zeros(N_NODES + 1, dtype=np.int64)
    np.cumsum(deg, out=starts[1:])
    srcs = src[esort]
    rank = np.arange(N_EDGES, dtype=np.int64) - starts[srcs]
    q = rank // 2                     # pair (level) index
    par = rank % 2
    colc = lbase[q] + s_pos[srcs] // 2
    halfq = s_pos[srcs] % 2
    slot = colc * 4 + halfq * 2 + par
    core_of_edge = srcs // NC_NODES

    # fp8 attr, pre-scaled by 32/deg (relu is positively homogeneous)
    recall = (32.0 / np.maximum(deg, 1)).astype(np.float32)
    ea = np.clip(edge_attr * recall[src][:, None], -240.0, 240.0).astype(NPF8)

    # shared weights: w2 = W_time.T replicated per 32-partition slot
    w2 = np.concatenate([W_time.T] * 4, axis=0).astype(NPF8)            # [128,64]
    wd = np.concatenate([W_down.T[0:128], W_down.T[128:256]], axis=1).astype(NPBF)
    wfT = W_fusion.T.copy()                                             # [128, 64]
    wfT[64:128] *= 1.0 / 32.0         # compensate the 32/deg pre-scale
    # fin = [ntf(0:64); nf(64:128)] for both parities
    wf = np.concatenate([wfT[64:128], wfT[0:64]], axis=0).astype(NPBF)  # [128,64]
    wu = np.concatenate([W_up.T, W_up.T], axis=0).astype(NPBF)          # [128,256]
    fold = np.concatenate([np.eye(64), np.eye(64)], axis=1)             # [64,128]
    fold = np.concatenate([fold, fold], axis=0).astype(NPF8)            # [128,128]
    biases = np.zeros((128, 4), dtype=np.float32)
    biases[0:64, 0] = b_down
    biases[64:128, 0] = b_down
    biases[0:64, 1] = b_fusion
    biases[64:128, 1] = b_fusion
    biases[:, 2] = b_up[0:128]
    biases[:, 3] = b_up[128:256]

    in_maps = []
    node_cols = np.empty((NCORES, NC_NODES), dtype=np.int64)
    for c in range(NCORES):
        em = core_of_edge == c
        attr_flat = np.zeros((NCH, C_CH * 4, EDGE_DIM), dtype=NPF8)
        ech = chn[srcs[em]]
        attr_flat[ech, slot[em]] = ea[esort[em]]
        # [NCH, C*4, 32] -> per chunk [128, C]: partition = slot*32+dim
        attr2 = np.ascontiguousarray(
            attr_flat.reshape(NCH, C_CH, 4, EDGE_DIM)
            .transpose(0, 2, 3, 1).reshape(NCH * 128, C_CH))

        # x layout: per chunk cols [e_h0 | o_h0 | e_h1 | o_h1]
        nlo = c * NC_NODES
        lpos = s_pos[nlo:nlo + NC_NODES]
        lch = chn[nlo:nlo + NC_NODES]
        colq = lpos // 2
        colp = lpos % 2
        node_cols[c] = lch * XT_COLS * 0 + colp * BLK + colq  # col within h-block
        xt = np.zeros((NCH * 128, XT_COLS), dtype=np.float32)
        xv = x[nlo:nlo + NC_NODES]                            # [6250, 256]
        rows = lch * 128
        # scatter x into layout
        for h in range(2):
            base = 2 * BLK * h
            cols = base + colp * BLK + colq
            for d in range(128):
                xt[rows + d, cols] = xv[:, 128 * h + d]
        in_maps.append({
            "attr2": attr2,
            "xt": xt.astype(NPBF),
            "w2": w2,
            "wd": wd,
            "wf": wf,
            "wu": wu,
            "fold": fold,
            "biases": biases,
        })
    return caps, in_maps, node_cols


def run(caps, in_maps, trace=False, **kw):
    nc = _build_graph(caps)
    return run_bass_kernel_spmd(nc, in_maps, core_ids=list(range(NCORES)),
                                trace=trace, **kw)


def unshard(results, node_cols):
    out = np.empty((N_NODES, IN_CH), dtype=np.float32)
    lch = np.minimum(np.arange(NC_NODES) // CHUNK, NCH - 1)
    rows = lch * 128
    for c in range(NCORES):
        o = np.asarray(results[c]["out"], dtype=np.float32)   # [NCH*128, XT_COLS]
        cols = node_cols[c]
        for h in range(2):
            colh = 2 * BLK * h + cols
            for d in range(128):
                out[c * NC_NODES:(c + 1) * NC_NODES, 128 * h + d] = \
                    o[rows + d, colh]
    return out


def kernel(**inputs):
    caps, in_maps, node_cols = prepare(**inputs)
    res = run(caps, in_maps, trace=False)
    return unshard(res.results, node_cols)
